# revision 7
# baseline (speedup 1.0000x reference)
"""Trainium2 Bass kernel for nn_AttentionBranch (sparse GQA attention + RoPE).

Problem (hardcoded): B=1, S=2176, 32 q heads, 8 kv heads, head_dim=128,
mask = causal & (sliding-window-256 | kv < 128 meta prefix), fp32 io.

Sharding: 8 cores; core c owns q heads [4c, 4c+4) and kv head c (GQA group).

Per-core dataflow (SPMD, one Bass program):
  - RoPE on-device: the two products on DVE, the add pass on the otherwise-
    idle GPSIMD engine.  k/q0/q1 ship straight-only; their sin-product is
    done as two 64-partition DVE ops reading the opposite half in place
    (no swapped copy at all).  q2/q3 arrive late enough that their host-
    made swapped copies ride the idle tail of the input stream, saving a
    DVE pass each.
  - Input DMAs are batched and ordered by first use: k+cos+sin ship as ONE
    [3, D, S] tensor chunked in 6 column spans.  Each dma_start costs
    ~600ns of sync-sequencer issue time, so fewer, fatter DMAs shorten the
    ramp; V splits in two so the first PV never waits on the full table.
  - Block-sparse attention over 128-row q blocks: kv blocks {0, i-2, i-1, i}.
    Work is organized in 3-q-block pieces; per piece all QK scores (<=1536
    cols, kv on partitions) go into ONE 3-bank PSUM group tile via QK
    matmuls + triangular additive masks (matmul accumulate), then ONE exp
    (ACT) per piece -> pb bf16.
  - PV+Z fused: V ships with a ones column appended ([kv, NB, 129]); one
    probs-stationary matmul per (strip, 128-col q sub-block) produces
    out[q, dv 0..127] AND the softmax denominator Z in column 128 of the
    same PSUM accumulator.
  - Normalize per piece: strided reciprocal of the Z columns + one
    broadcast tensor_mul; output written bf16 into a 2-piece osb group
    tile, one store DMA per group (12 stores instead of 24).
  - Pieces are pipelined with a 2-piece lag; PSUM: 2x3-bank group tiles +
    2x1-bank out accumulators = 8 banks.  RoPE for later heads is emitted
    two units ahead of need so QK never stalls on a rope semaphore (PE
    p-state halves for ~3us after any idle gap).
"""

import math
import os
from functools import lru_cache

import numpy as np
import ml_dtypes

S = 2176
D = 128
NB = S // 128  # 17 q/kv blocks
HQ_PER_CORE = 4
N_CORES = 8
WINDOW = 256
META = 128
ROPE_BASE = 10000.0
SCALE = 1.0 / math.sqrt(D)

BF16 = ml_dtypes.bfloat16
LAST_RESULT = None

# q-block ranges of the per-head processing pieces. 3 blocks/piece so the
# piece's out+Z accumulator (3 x 129 f32 = 387 cols) fits ONE PSUM bank.
PIECES = [(0, 2), (3, 5), (6, 8), (9, 11), (12, 14), (15, 16)]

# input chunk column spans (kcs rope granularity)
CHUNKS = [(0, 384), (384, 768), (768, 1152), (1152, 1536), (1536, 1920),
          (1920, S)]
Q0_CHUNKS = [(0, 384), (384, 1152), (1152, 1536), (1536, S)]
Q1_CHUNKS = [(0, 1152), (1152, S)]


def _strips_for_piece(b0, b1):
    """Work list for q-blocks [b0, b1]. Each strip is one kv-block (or meta
    chunk) x a contiguous span of q columns."""
    strips = []
    lo_col = b0 * 128
    hi_col = (b1 + 1) * 128
    # meta chunk: kv block 0, dense except causal diag for q-block 0.
    col = lo_col
    while col < hi_col:
        span = min(512, hi_col - col)
        strips.append(
            dict(
                kvblk=0,
                qlo=col,
                qhi=col + span,
                meta=True,
                diag_u=0 if col == 0 else None,
                i2_u=None,
            )
        )
        col += span
    # window strips: kv block j covers q blocks {j, j+1, j+2} (j >= 1).
    for j in range(1, NB):
        i0 = max(j, b0)
        i1 = min(j + 2, b1)
        if i0 > i1:
            continue
        strips.append(
            dict(
                kvblk=j,
                qlo=i0 * 128,
                qhi=(i1 + 1) * 128,
                meta=False,
                diag_u=0 if i0 == j else None,
                i2_u=(i1 - i0) * 128 if i1 == j + 2 else None,
            )
        )
    return strips


def _pack_group(strips):
    """Hole-free packing of a piece's strips into one <=1536-col group tile
    such that every strip stays inside one 512-col PSUM bank. Sets
    st['goff']; returns total cols. Meta strip is packed (and emitted)
    first so its PV+Z matmuls open each q-block's accumulation chain."""

    def span(s):
        return s["qhi"] - s["qlo"]

    rest = sorted(strips, key=lambda s: (-span(s), not s["meta"]))
    order, fill = [], 0
    while rest:
        pick = None
        for s in rest:
            sp = span(s)
            if fill + sp <= 1536 and (fill % 512) + sp <= 512:
                pick = s
                break
        assert pick is not None, "packing stuck"
        rest.remove(pick)
        pick["goff"] = fill
        fill += span(pick)
        order.append(pick)
    assert order[0]["meta"]
    return order, fill


@lru_cache(maxsize=1)
def _build_program():
    import concourse.bass as bass
    import concourse.mybir as mybir
    import concourse.tile as tile
    from concourse import bacc

    bf = mybir.dt.bfloat16
    f32 = mybir.dt.float32
    EXP = mybir.ActivationFunctionType.Exp

    nc = bacc.Bacc(None)

    # kcs planes: 0 = k straight, 1 = cos, 2 = sin (sign-folded).  One DMA
    # per column chunk covers all three planes.
    kcs_d = nc.declare_dram_parameter("kcs", [3, D, S], bf, isOutput=False)
    q0_d = nc.declare_dram_parameter("q0", [D, S], bf, isOutput=False)
    # qr planes: q1, q2, q2 swapped, q3, q3 swapped
    qr_d = nc.declare_dram_parameter("qr", [5, D, S], bf, isOutput=False)
    v_d = nc.declare_dram_parameter("v", [D, NB, 129], bf, isOutput=False)
    msk_d = nc.declare_dram_parameter("msk", [D, 3, 128], bf, isOutput=False)
    out_d = nc.declare_dram_parameter("out", [HQ_PER_CORE, NB, D, 128], bf, isOutput=True)

    with tile.TileContext(nc) as tc:
        with (
            tc.tile_pool(name="persist", bufs=1) as persist,
            tc.tile_pool(name="probs", bufs=4) as probs_pool,
            tc.tile_pool(name="norm", bufs=4) as norm_pool,
            tc.tile_pool(name="osb", bufs=4) as osb_pool,
            tc.tile_pool(name="grp", bufs=2, space="PSUM") as grp_psum,
            tc.tile_pool(name="acc", bufs=2, space="PSUM") as acc_psum,
        ):
            kcs = persist.tile([D, 3, S], bf)
            qt0 = persist.tile([D, S], bf)
            qr = persist.tile([D, 5, S], bf)
            vt = persist.tile([D, NB, 129], bf)
            msk = persist.tile([D, 3, 128], bf)
            ones = persist.tile([D, 128], bf)
            ropek = persist.tile([D, S], bf)
            ropeq = persist.tile([D, HQ_PER_CORE, S], bf)
            # sin-product temps: plane 0 for k, 1/2 for odd/even q heads
            ropet = persist.tile([D, 3, S], bf)
            # half-swapped copies of the straight-only tensors (k, q0, q1),
            # built by partition-offset DVE copies (4x single-src mode)
            swp = persist.tile([D, 3, S], bf)

            kcsr = kcs_d.rearrange("s d t -> d s t")

            # Input DMA order: head-0's first-piece data first, then the
            # remaining chunks interleaved so each lands just before use.
            def dma_kcs(ci):
                lo, hi = CHUNKS[ci]
                nc.sync.dma_start(out=kcs[:, :, lo:hi], in_=kcsr[:, :, lo:hi])

            def dma_q0(ci):
                lo, hi = Q0_CHUNKS[ci]
                nc.sync.dma_start(out=qt0[:, lo:hi], in_=q0_d[:, lo:hi])

            dma_kcs(0)
            dma_q0(0)
            nc.sync.dma_start(out=msk, in_=msk_d[:])
            nc.sync.dma_start(out=vt[:, 0:6], in_=v_d[:, 0:6])
            dma_kcs(1)
            dma_q0(1)
            dma_kcs(2)
            nc.sync.dma_start(out=vt[:, 6:NB], in_=v_d[:, 6:NB])
            dma_q0(2)
            dma_kcs(3)
            dma_q0(3)
            lo, hi = Q1_CHUNKS[0]
            nc.sync.dma_start(out=qr[:, 0, lo:hi], in_=qr_d[0, :, lo:hi])
            dma_kcs(4)
            dma_kcs(5)
            lo, hi = Q1_CHUNKS[1]
            nc.sync.dma_start(out=qr[:, 0, lo:hi], in_=qr_d[0, :, lo:hi])
            qrr = qr_d.rearrange("s d t -> d s t")
            nc.sync.dma_start(out=qr[:, 1:3], in_=qrr[:, 1:3])
            nc.sync.dma_start(out=qr[:, 3:5], in_=qrr[:, 3:5])

            nc.vector.memset(ones, 1.0)

            # trigger the exp ACT-table load early (off the critical path)
            tldw = norm_pool.tile([D, 3, 1], f32, tag="rz")
            nc.scalar.activation(tldw[:, 0], ones[:, :1], EXP)

            # PE warm-up: DMA-independent dummy matmuls keep the HAM
            # activity window busy so the real stream starts ramped.
            wz = acc_psum.tile([D, 512], f32, tag="ot")
            for _ in range(20):
                nc.tensor.matmul(
                    wz[:, :128], lhsT=ones, rhs=ones, start=True, stop=True
                )

            def swap_into(plane, src, lo, hi):
                """swp[:, plane] = half-swap(src): two partition-offset
                single-source DVE copies."""
                nc.vector.tensor_copy(swp[0:64, plane, lo:hi], src[64:128, lo:hi])
                nc.vector.tensor_copy(swp[64:128, plane, lo:hi], src[0:64, lo:hi])

            def rope_pair(dst, tmp, straight, swapped, lo, hi):
                """dst = straight*cos + swapped*sin on DVE(x2)+POOL add."""
                sl = slice(lo, hi)
                nc.vector.tensor_mul(dst[:, sl], straight[:, sl], kcs[:, 1, sl])
                nc.vector.tensor_mul(tmp[:, sl], swapped[:, sl], kcs[:, 2, sl])
                nc.gpsimd.tensor_add(dst[:, sl], dst[:, sl], tmp[:, sl])

            def rope_k(lo, hi):
                swap_into(0, kcs[:, 0], lo, hi)
                rope_pair(ropek, ropet[:, 0], kcs[:, 0], swp[:, 0], lo, hi)

            def rope_q(h, lo, hi):
                tmp = ropet[:, 1 + (h & 1)]
                if h == 0:
                    swap_into(1, qt0, lo, hi)
                    rope_pair(ropeq[:, 0], tmp, qt0, swp[:, 1], lo, hi)
                elif h == 1:
                    swap_into(2, qr[:, 0], lo, hi)
                    rope_pair(ropeq[:, 1], tmp, qr[:, 0], swp[:, 2], lo, hi)
                elif h == 2:
                    rope_pair(ropeq[:, 2], tmp, qr[:, 1], qr[:, 2], lo, hi)
                else:
                    rope_pair(ropeq[:, 3], tmp, qr[:, 3], qr[:, 4], lo, hi)

            def emit_qk(h, st, gp):
                """QK + additive-mask matmuls for one strip into the group
                tile at st['goff']."""
                span = st["qhi"] - st["qlo"]
                go = st["goff"]
                masks = []
                if st["diag_u"] is not None:
                    masks.append((st["diag_u"], 0))
                if st["i2_u"] is not None:
                    masks.append((st["i2_u"], 1))
                nc.tensor.matmul(
                    gp[:, go : go + span],
                    lhsT=ropek[:, st["kvblk"] * 128 : (st["kvblk"] + 1) * 128],
                    rhs=ropeq[:, h, st["qlo"] : st["qhi"]],
                    start=True,
                    stop=not masks,
                )
                for mi, (u, g) in enumerate(masks):
                    nc.tensor.matmul(
                        gp[:, go + u : go + u + 128],
                        lhsT=msk[:, 2],
                        rhs=msk[:, g],
                        start=False,
                        stop=mi == len(masks) - 1,
                    )

            def emit_piece_back(work):
                """PV+Z matmuls for all strips of a piece, then normalize
                into the 2-piece osb group tile; store once per group."""
                h, pidx, b0, b1, order, pbg, otq, osb = work
                nq = b1 - b0 + 1
                last_for_qb = {}
                for si, st in enumerate(order):
                    for qb in range(st["qlo"] // 128, st["qhi"] // 128):
                        last_for_qb[qb] = si
                # start=True ONLY on the very first matmul into the bank: it
                # clears has_written for the WHOLE bank, so later first-
                # writes per q-block chain must use start=False (overwrite-
                # where-bit-unset initializes them correctly).
                for si, st in enumerate(order):
                    for k in range((st["qhi"] - st["qlo"]) // 128):
                        qb = st["qlo"] // 128 + k
                        b = qb - b0
                        nc.tensor.matmul(
                            otq[:, b],
                            lhsT=pbg[:, st["goff"] + 128 * k : st["goff"] + 128 * (k + 1)],
                            rhs=vt[:, st["kvblk"]],
                            start=si == 0 and k == 0,
                            stop=last_for_qb[qb] == si,
                            skip_group_check=True,
                        )
                # normalize: strided recip of the nq Z columns, then one
                # dv-broadcast multiply; write bf16 into the group tile.
                sub = pidx % 2
                rzt = norm_pool.tile([D, 3, 1], f32, tag="rz")
                nc.vector.reciprocal_approx_fast(
                    rzt[:, :nq], otq[:, :nq, 128:129]
                )
                nc.vector.tensor_mul(
                    osb[:, 3 * sub : 3 * sub + nq],
                    otq[:, :nq, :128],
                    rzt[:, :nq].broadcast_to([D, nq, 128]),
                )
                if sub == 1 or pidx == 5:
                    g0 = (pidx // 2) * 6  # first q-block of the group
                    gn = 3 * sub + nq
                    orh = out_d[h].rearrange("j p v -> p j v")
                    nc.sync.dma_start(
                        out=orh[:, g0 : g0 + gn], in_=osb[:, :gn]
                    )

            # Software-pipelined emission with a lag: PE runs QK of later
            # pieces while ACT computes earlier pieces' exps.  Rope for
            # each unit is emitted two units AHEAD of need so QK matmuls
            # never wait on a rope semaphore.
            from collections import deque

            order = [(h, p) for h in range(HQ_PER_CORE) for p in range(6)]
            # Phase-A DVE/pool priming: head-0 + k rope gated only on the
            # (in-order) input chunk arrivals.
            ki = iter(CHUNKS)
            qi = iter(Q0_CHUNKS)
            for a, b in ((0, 0), (1, 1), (2, 2), (3, 3), (4, None), (5, None)):
                lo, hi = CHUNKS[a]
                rope_k(lo, hi)
                if b is not None:
                    lo, hi = Q0_CHUNKS[b]
                    rope_q(0, lo, hi)
            ropeq_done = [S, 0, 0, 0]
            pending = deque()
            osb_tiles = {}

            def ensure_rope(h, pidx):
                """Emit rope so that unit (h, pidx) can run."""
                b0, b1 = PIECES[pidx]
                need = (b1 + 1) * 128
                if ropeq_done[h] < need:
                    lo = ropeq_done[h]
                    rope_q(h, lo, need)
                    ropeq_done[h] = need

            for ui, (h, pidx) in enumerate(order):
                ensure_rope(h, pidx)
                for la in (1, 2):
                    if ui + la < len(order):
                        ensure_rope(*order[ui + la])
                LAG = 1 if ui < 4 else 2
                b0, b1 = PIECES[pidx]
                strips, gcols = _pack_group(_strips_for_piece(b0, b1))
                gp = grp_psum.tile([D, 1536], f32, tag="gp")
                pbg = probs_pool.tile([D, 1536], bf, tag="pb")
                otq = acc_psum.tile([D, 3, 129], f32, tag="ot")
                if pidx % 2 == 0:
                    osb = osb_pool.tile([D, 6, 128], bf, tag="osb")
                    osb_tiles[h] = osb
                osb = osb_tiles[h]
                for st in strips:
                    emit_qk(h, st, gp)
                nc.scalar.activation(
                    pbg[:, :gcols], gp[:, :gcols], EXP, scale=SCALE
                )
                pending.append((h, pidx, b0, b1, strips, pbg, otq, osb))
                while len(pending) > LAG:
                    emit_piece_back(pending.popleft())
            while pending:
                emit_piece_back(pending.popleft())

    nc.finalize()
    return nc


@lru_cache(maxsize=1)
def _rope_tables():
    inv_freq = 1.0 / (ROPE_BASE ** (np.arange(0, D, 2, dtype=np.float64) / D))
    pos = np.arange(S, dtype=np.float64)
    freqs = pos[:, None] * inv_freq[None, :]  # [S, 64]
    emb = np.concatenate([freqs, freqs], axis=-1)  # [S, D]
    # match the f32 reference: compute cos/sin at f32 granularity
    cosT = np.cos(emb.astype(np.float32)).T.astype(np.float32)  # [D, S]
    sinT = np.sin(emb.astype(np.float32)).T.astype(np.float32)
    sinTpm = np.concatenate([-sinT[:64], sinT[64:]], axis=0)
    return cosT, sinTpm


def _mask_tiles():
    """[128, 3, 128]: additive score masks (0 keep / -1e30 drop) for the
    causal-diag and window-tail blocks, plus a 128x128 identity (the
    stationary operand of the mask-accumulate matmuls)."""
    c = np.arange(128)[:, None]
    u = np.arange(128)[None, :]
    a_diag = np.where(u >= c, 0.0, -1e30).astype(np.float32)
    a_tail = np.where(u <= c, 0.0, -1e30).astype(np.float32)
    ident = np.eye(128, dtype=np.float32)
    return np.stack([a_diag, a_tail, ident], axis=1)  # [128, 3, 128]


def _swap_halves(xT):
    return np.concatenate([xT[64:], xT[:64]], axis=0)


def _install_ntff_shim():
    """Provide antenv.axon_hooks (NTFF profile hook) if the image lacks it,
    so run_bass_kernel_spmd(trace=True) can capture HW profiles via the
    axon PJRT .so. Silently no-ops if unavailable."""
    import sys
    import types

    try:
        from antenv.axon_hooks import get_axon_ntff_profile_hook  # noqa: F401

        return
    except ImportError:
        pass
    try:
        import contextlib
        import ctypes

        lib = ctypes.CDLL("/opt/axon/libaxon_pjrt.so")
        if not hasattr(lib, "axon_start_nrt_profile"):
            return
        lib.axon_start_nrt_profile.argtypes = [
            ctypes.POINTER(ctypes.c_int64),
            ctypes.c_size_t,
        ]
        lib.axon_start_nrt_profile.restype = ctypes.c_int64
        lib.axon_stop_nrt_profile.argtypes = [ctypes.c_char_p]
        lib.axon_stop_nrt_profile.restype = ctypes.c_int64

        @contextlib.contextmanager
        def _hook(output_dir, device_ids):
            import jax

            jax.devices()
            if device_ids:
                ids = (ctypes.c_int64 * len(device_ids))(*device_ids)
                rc = lib.axon_start_nrt_profile(ids, len(device_ids))
            else:
                rc = lib.axon_start_nrt_profile(None, 0)
            if rc != 0:
                raise RuntimeError(f"axon_start_nrt_profile rc={rc}")
            try:
                yield
            finally:
                n = lib.axon_stop_nrt_profile(str(output_dir).encode())
                print(f"ntff profile: {n} file(s) -> {output_dir}", file=sys.stderr)

        mod = types.ModuleType("antenv.axon_hooks")
        mod._hook = _hook
        mod.get_axon_ntff_profile_hook = lambda: _hook
        mod.set_axon_ntff_profile_hook = lambda h: setattr(mod, "_hook", h)
        import antenv

        antenv.axon_hooks = mod
        sys.modules["antenv.axon_hooks"] = mod
    except Exception:
        pass


def kernel(query_states, key_states, value_states):
    from concourse.bass_utils import run_bass_kernel_spmd

    _install_ntff_shim()

    nc = _build_program()

    q = np.asarray(query_states)[0]  # [S, 4096]
    k = np.asarray(key_states)[0]  # [S, 1024]
    v = np.asarray(value_states)[0]  # [S, 1024]

    cosT, sinTpm = _rope_tables()
    msk = _mask_tiles().astype(BF16)

    in_maps = []
    for c in range(N_CORES):
        kh = np.ascontiguousarray(k[:, c * D : (c + 1) * D].T)  # [D, S]
        kcs = np.stack([kh, cosT, sinTpm], axis=0).astype(BF16)  # [3, D, S]
        q0 = np.ascontiguousarray(
            q[:, 4 * c * D : (4 * c + 1) * D].T
        ).astype(BF16)
        qr = np.empty((5, D, S), dtype=BF16)
        heads = [4 * c + 1, 4 * c + 2, 4 * c + 3]
        q1h = np.ascontiguousarray(q[:, heads[0] * D : (heads[0] + 1) * D].T)
        q2h = np.ascontiguousarray(q[:, heads[1] * D : (heads[1] + 1) * D].T)
        q3h = np.ascontiguousarray(q[:, heads[2] * D : (heads[2] + 1) * D].T)
        qr[0] = q1h.astype(BF16)
        qr[1] = q2h.astype(BF16)
        qr[2] = _swap_halves(q2h).astype(BF16)
        qr[3] = q3h.astype(BF16)
        qr[4] = _swap_halves(q3h).astype(BF16)
        vh = v[:, c * D : (c + 1) * D]  # [S, D]
        vts = np.ones((D, NB, 129), dtype=BF16)
        vts[:, :, :128] = vh.reshape(NB, 128, D).transpose(1, 0, 2).astype(BF16)
        in_maps.append({"kcs": kcs, "q0": q0, "qr": qr, "v": vts, "msk": msk})

    res = run_bass_kernel_spmd(nc, in_maps, core_ids=list(range(N_CORES)))
    global LAST_RESULT
    LAST_RESULT = res

    out = np.empty((S, 32, D), dtype=np.float32)
    for c in range(N_CORES):
        o = np.asarray(res.results[c]["out"], dtype=np.float32)  # [4, NB, D, 128]
        for hh in range(HQ_PER_CORE):
            out[:, 4 * c + hh, :] = o[hh].reshape(S, D)
    return out.reshape(1, S, 32 * D)
